# revision 7
# baseline (speedup 1.0000x reference)
"""Multi-head differential attention on 8 TRN2 NeuronCores.

Strategy (tensor parallel over heads + sequence-parallel epilogue):
  - Each core owns 2 of the 16 heads. It computes Q/K/V projections for its
    heads (from a shared on-chip X^T), flash-style attention entirely on-chip
    in transposed layouts, and the per-head differential-softmax combine.
  - An AllToAll (1 MB/rank) redistributes the combined per-head outputs
    Ocat^T [c, q] from head-sharding to sequence-sharding.
  - Each core then does its 256-row slice of Y = Ocat @ Wo + bo, LayerNorm,
    and the final (1 - lambda_init) scale. Host concatenates the 8 slices.

All matmuls run in float32r (full PE rate at >=256 free dim, ~13-bit mantissa);
exp/softmax/LN math in fp32 on ACT/DVE.
"""
import numpy as np

import concourse.bass as bass
import concourse.mybir as mybir
import concourse.tile as tile
from concourse.bass_utils import run_bass_kernel_spmd
import bass_rust

F32 = mybir.dt.float32
F32R = mybir.dt.float32r

B, S, E, H, D = 1, 2048, 1024, 16, 64
NCORES = 8
HC = H // NCORES          # heads per core = 2
SS = S // NCORES          # sequence shard = 256
C2D = 2 * D               # 128
LAM_INIT = 0.8
LN_EPS = 1e-5
QCH = 512                 # q-chunk width in the attention loop
NQC = S // QCH            # 4
NKB = S // 128            # 16 k-blocks
NEC = E // 128            # 8 contraction chunks
SCALE = 1.0 / float(np.sqrt(D))

_CACHE = {}


def _split_excess_waits(nc, max_waits=1):
    """walrus in this env only accepts ~1 sync wait per instruction; move the
    rest onto same-engine InstNoOp carriers emitted just before."""
    for bb in nc.main_func.blocks:
        new_list = []
        for ins in bb.instructions:
            w = list(ins.sync_info.on_wait) if ins.sync_info else []
            if len(w) > max_waits:
                extra, keep = w[:-max_waits], w[-max_waits:]
                for i, sw in enumerate(extra):
                    nop = mybir.InstNoOp(
                        name=f"{ins.name}-ps{i}", ins=[], outs=[],
                        engine=ins.engine,
                        sync_info=bass_rust.SyncInfo(on_wait=[sw], on_update=[]),
                    )
                    nc.register_instruction(nop)
                    new_list.append(nop)
                ins.sync_info = bass_rust.SyncInfo(
                    on_wait=keep, on_update=list(ins.sync_info.on_update))
            new_list.append(ins)
        bb.instructions[:] = new_list


def _dram_bcast(ap, parts):
    """Partition-broadcast read AP over an ExternalInput DRAM tensor."""
    return bass.AP(tensor=ap.tensor, offset=ap.offset,
                   ap=[[0, parts]] + [list(d) for d in ap.ap])


def build_program():
    nc = bass.Bass()

    X = nc.declare_dram_parameter("X", [S, E], F32R, isOutput=False)
    WQ = nc.declare_dram_parameter("WQ", [HC, E, C2D], F32R, isOutput=False)
    WK = nc.declare_dram_parameter("WK", [HC, E, C2D], F32R, isOutput=False)
    WV = nc.declare_dram_parameter("WV", [HC, E, D], F32R, isOutput=False)
    BQ = nc.declare_dram_parameter("BQ", [HC, C2D], F32, isOutput=False)
    BK = nc.declare_dram_parameter("BK", [HC, C2D], F32, isOutput=False)
    BV = nc.declare_dram_parameter("BV", [HC * D], F32, isOutput=False)
    WO = nc.declare_dram_parameter("WO", [E, E], F32R, isOutput=False)
    BO = nc.declare_dram_parameter("BO", [E], F32, isOutput=False)
    GAMMA = nc.declare_dram_parameter("GAMMA", [E], F32, isOutput=False)
    BETA = nc.declare_dram_parameter("BETA", [E], F32, isOutput=False)
    LAM = nc.declare_dram_parameter("LAM", [1], F32, isOutput=False)
    IDENT = nc.declare_dram_parameter("IDENT", [128, 128], F32R, isOutput=False)
    ONESROW = nc.declare_dram_parameter("ONESROW", [1, 64], F32R, isOutput=False)
    ONESCOL = nc.declare_dram_parameter("ONESCOL", [128, 1], F32R, isOutput=False)
    YOUT = nc.declare_dram_parameter("YOUT", [SS, E], F32, isOutput=True)

    with tile.TileContext(nc) as tc:
        with (
            tc.tile_pool(name="singles", bufs=1) as singles,
            tc.tile_pool(name="persist", bufs=1) as persist,
            tc.tile_pool(name="dram", bufs=1, space="DRAM") as dram,
        ):
            # ---------- constants ----------
            ident = singles.tile([128, 128], F32R)
            nc.sync.dma_start(out=ident[:], in_=IDENT[:, :])
            onesrow = singles.tile([1, 64], F32R)
            nc.gpsimd.dma_start(out=onesrow[:], in_=ONESROW[:, :])
            onescol = singles.tile([128, 1], F32R)
            nc.gpsimd.dma_start(out=onescol[:], in_=ONESCOL[:, :])
            lam_b = singles.tile([128, 1], F32)
            nc.gpsimd.dma_start(out=lam_b[:], in_=_dram_bcast(LAM[:], 128))
            gb2 = singles.tile([128, E], F32)
            nc.gpsimd.dma_start(out=gb2[:], in_=_dram_bcast(GAMMA[:], 128))
            bb2 = singles.tile([128, E], F32)
            nc.gpsimd.dma_start(out=bb2[:], in_=_dram_bcast(BETA[:], 128))
            bo_b = singles.tile([128, E], F32)
            nc.gpsimd.dma_start(out=bo_b[:], in_=_dram_bcast(BO[:], 128))
            # fold the final (1 - LAM_INIT) scale into gamma/beta broadcasts
            nc.vector.tensor_scalar_mul(gb2[:], gb2[:], 1.0 - LAM_INIT)
            nc.vector.tensor_scalar_mul(bb2[:], bb2[:], 1.0 - LAM_INIT)
            eps_t = singles.tile([128, 1], F32)
            nc.gpsimd.memset(eps_t[:], LN_EPS)

            bq_t, bk_t = [], []
            for h in range(HC):
                t = singles.tile([128, 1], F32, name=f"bq{h}", tag=f"bq{h}")
                nc.gpsimd.dma_start(out=t[:], in_=BQ[h, :].unsqueeze(1))
                bq_t.append(t)
                t = singles.tile([128, 1], F32, name=f"bk{h}", tag=f"bk{h}")
                nc.gpsimd.dma_start(out=t[:], in_=BK[h, :].unsqueeze(1))
                bk_t.append(t)
            bv_t = singles.tile([128, 1], F32)
            nc.gpsimd.dma_start(out=bv_t[:], in_=BV[:].unsqueeze(1))

            # long-lived attention operands
            QT = [persist.tile([128, S], F32R, name=f"QT{h}", tag=f"QT{h}")
                  for h in range(HC)]
            K1z = [persist.tile([128, S], F32R, name=f"K1z{h}", tag=f"K1z{h}")
                   for h in range(HC)]
            K2z = [persist.tile([128, S], F32R, name=f"K2z{h}", tag=f"K2z{h}")
                   for h in range(HC)]
            Vex1 = [persist.tile([128, NKB * 65], F32R, name=f"Vex1{h}",
                                 tag=f"Vex1{h}") for h in range(HC)]
            Vex2 = [persist.tile([128, NKB * 65], F32R, name=f"Vex2{h}",
                                 tag=f"Vex2{h}") for h in range(HC)]

            # one AllToAll per head so the first overlaps the second head's
            # attention compute
            a2a_in = [dram.tile([NCORES, D, SS], F32, name=f"a2ai{h}",
                                tag=f"a2ai{h}") for h in range(HC)]
            a2a_out = [dram.tile([NCORES, D, SS], F32, name=f"a2ao{h}",
                                 tag=f"a2ao{h}") for h in range(HC)]

            # ================= projection scope (xT lives here) =============
            with (
                tc.tile_pool(name="xtp", bufs=1) as xtp,
                tc.tile_pool(name="xstage", bufs=3) as xstage,
                tc.tile_pool(name="psT", bufs=4, space="PSUM") as psT,
                tc.tile_pool(name="psP", bufs=2, space="PSUM") as psP,
            ):
                # projection weights: [128 part (e within chunk), NEC, cols]
                wq_t = [xtp.tile([128, NEC, C2D], F32R, name=f"wq{h}",
                                 tag=f"wq{h}") for h in range(HC)]
                wk_t = [xtp.tile([128, NEC, C2D], F32R, name=f"wk{h}",
                                 tag=f"wk{h}") for h in range(HC)]
                for h in range(HC):
                    nc.gpsimd.dma_start(
                        out=wq_t[h][:],
                        in_=WQ[h].rearrange("(a p) c -> p a c", p=128))
                    nc.gpsimd.dma_start(
                        out=wk_t[h][:],
                        in_=WK[h].rearrange("(a p) c -> p a c", p=128))
                wv_t = xtp.tile([128, NEC, 2 * D], F32R)
                for h in range(HC):
                    nc.gpsimd.dma_start(
                        out=wv_t[:, :, h * D:(h + 1) * D],
                        in_=WV[h].rearrange("(a p) c -> p a c", p=128))

                # ----- X^T via PE transposes -----
                xT = [xtp.tile([128, S], F32R, name=f"xT{ec}", tag=f"xT{ec}")
                      for ec in range(NEC)]
                for sc in range(S // 128):
                    xs = xstage.tile([128, E], F32R, name="xs")
                    nc.sync.dma_start(out=xs[:],
                                      in_=X[sc * 128:(sc + 1) * 128, :])
                    for ec in range(NEC):
                        pst = psT.tile([128, 128], F32R, name="pst")
                        nc.tensor.transpose(
                            pst[:], xs[:, ec * 128:(ec + 1) * 128], ident[:])
                        nc.vector.tensor_copy(
                            xT[ec][:, sc * 128:(sc + 1) * 128], pst[:])

                # zero the unused halves of K1z/K2z once
                for h in range(HC):
                    nc.vector.tensor_scalar_mul(
                        K1z[h][64:128, :],
                        onescol[0:64, :].bitcast(F32).to_broadcast((64, S)), 0.0)
                    nc.vector.tensor_scalar_mul(
                        K2z[h][0:64, :],
                        onescol[0:64, :].bitcast(F32).to_broadcast((64, S)), 0.0)

                # ----- projections (transposed outputs) -----
                VTp = xtp.tile([128, S], F32R, name="VTp")

                def project(dst_writes, w_tile, qc):
                    pq = psP.tile([128, QCH], F32, name="pq")
                    for ec in range(NEC):
                        nc.tensor.matmul(
                            pq[:], lhsT=w_tile[:, ec, :],
                            rhs=xT[ec][:, qc * QCH:(qc + 1) * QCH],
                            start=(ec == 0), stop=(ec == NEC - 1))
                    dst_writes(pq)

                for qc in range(NQC):
                    sl = slice(qc * QCH, (qc + 1) * QCH)
                    for h in range(HC):
                        def wq_writes(pq, h=h, sl=sl):
                            nc.vector.tensor_scalar(
                                out=QT[h][:, sl], in0=pq[:],
                                scalar1=bq_t[h][:], scalar2=None,
                                op0=mybir.AluOpType.add)
                        project(wq_writes, wq_t[h], qc)

                        def wk_writes(pq, h=h, sl=sl):
                            nc.vector.tensor_scalar(
                                out=K1z[h][0:64, sl], in0=pq[0:64, :],
                                scalar1=bk_t[h][0:64, :], scalar2=None,
                                op0=mybir.AluOpType.add)
                            nc.vector.tensor_scalar(
                                out=K2z[h][64:128, sl], in0=pq[64:128, :],
                                scalar1=bk_t[h][64:128, :], scalar2=None,
                                op0=mybir.AluOpType.add)
                        project(wk_writes, wk_t[h], qc)

                    def wv_writes(pq, sl=sl):
                        nc.vector.tensor_scalar(
                            out=VTp[:, sl], in0=pq[:], scalar1=bv_t[:],
                            scalar2=None, op0=mybir.AluOpType.add)
                    project(wv_writes, wv_t, qc)

                # ----- V natural + [V|1] / [lam*V|1] -----
                for h in range(HC):
                    for vx in (Vex1[h], Vex2[h]):
                        nc.vector.tensor_copy(
                            vx.rearrange("p (k c) -> p k c", c=65)[:, :, 64:65],
                            onescol[:].to_broadcast((128, NKB)).unsqueeze(2))
                for kb in range(NKB):
                    pvt = psT.tile([128, 128], F32R, name="pvt", tag="pst")
                    nc.tensor.transpose(
                        pvt[:], VTp[:, kb * 128:(kb + 1) * 128], ident[:])
                    for h in range(HC):
                        nc.vector.tensor_copy(
                            Vex1[h][:, kb * 65:kb * 65 + 64],
                            pvt[:, h * D:(h + 1) * D])
                        nc.vector.tensor_scalar(
                            out=Vex2[h][:, kb * 65:kb * 65 + 64],
                            in0=pvt[:, h * D:(h + 1) * D], scalar1=lam_b[:],
                            scalar2=None, op0=mybir.AluOpType.mult)
            # ============== end projection scope (xT space freed) ===========

            with (
                tc.tile_pool(name="phase4", bufs=1) as phase4,
                tc.tile_pool(name="spool", bufs=3) as spool,
                tc.tile_pool(name="epi", bufs=2) as epi,
            ):
                # Wo moving operand, loaded during attention:
                # [128 part (c within chunk), NEC, E]
                wo_t = phase4.tile([128, NEC, E], F32R)
                nc.gpsimd.dma_start(
                    out=wo_t[:], in_=WO[:, :].rearrange("(a p) e -> p a e",
                                                        p=128))

                # ---------- attention per (head, q-chunk) ----------
                NG = NKB // 2  # 2 k-blocks per exp group

                def emit_a2a(h):
                    nc.gpsimd.collective_compute(
                        "AllToAll", mybir.AluOpType.bypass,
                        replica_groups=[list(range(NCORES))],
                        ins=[a2a_in[h][:, :, :]],
                        outs=[a2a_out[h][:, :, :]],
                    )

                ocT = [phase4.tile([128, SS], F32R, name=f"ocT{cc}",
                                   tag=f"ocT{cc}") for cc in range(NCORES)]
                with (
                    tc.tile_pool(name="psA", bufs=1, space="PSUM") as psA,
                    tc.tile_pool(name="psO", bufs=2, space="PSUM") as psO,
                    tc.tile_pool(name="psB", bufs=1, space="PSUM") as psB,
                ):
                    pending = None  # deferred epilogue tail of prev iteration

                    def emit_tail(p):
                        h, qc, usb, rr = p
                        bp = psB.tile([64, 2 * QCH], F32, name="bp")
                        nc.tensor.matmul(bp[:, 0:QCH], lhsT=onesrow[:],
                                         rhs=rr[:, 0:QCH], start=True,
                                         stop=True)
                        nc.tensor.matmul(bp[:, QCH:2 * QCH], lhsT=onesrow[:],
                                         rhs=rr[:, QCH:2 * QCH], start=True,
                                         stop=True)
                        bsb = epi.tile([64, 2 * QCH], F32, name="bsb")
                        nc.vector.tensor_copy(bsb[:], bp[:])
                        tmp = epi.tile([64, QCH], F32, name="tmpc")
                        nc.vector.tensor_mul(tmp[:], usb[0:64, QCH:2 * QCH],
                                             bsb[:, QCH:2 * QCH])
                        oc = epi.tile([64, QCH], F32, name="oc")
                        nc.vector.tensor_mul(oc[:], usb[0:64, 0:QCH],
                                             bsb[:, 0:QCH])
                        nc.vector.tensor_sub(oc[:], oc[:], tmp[:])
                        # stage into a2a_in: q-chunk covers dests 2qc, 2qc+1
                        for half in range(2):
                            dest = 2 * qc + half
                            nc.gpsimd.dma_start(
                                out=a2a_in[h][dest, :, :],
                                in_=oc[:, half * SS:(half + 1) * SS])
                        if qc == NQC - 1:
                            emit_a2a(h)
                            for cc in range(NCORES):
                                nc.gpsimd.dma_start(
                                    out=ocT[cc][h * D:(h + 1) * D, :],
                                    in_=a2a_out[h][cc, :, :])

                    for h in range(HC):
                        for qc in range(NQC):
                            qsl = slice(qc * QCH, (qc + 1) * QCH)
                            o1 = psO.tile([65, QCH], F32, name="oacc",
                                          tag="oacc")
                            o2 = psO.tile([65, QCH], F32, name="oacc2",
                                          tag="oacc")
                            s_prev = None
                            for g in range(NG):
                                if g == NG - 2 and pending is not None:
                                    emit_tail(pending)
                                    pending = None
                                aq = psA.tile([128, 4, QCH], F32, name="aq",
                                              tag="aq")
                                for j in range(2):
                                    kb = 2 * g + j
                                    ksl = slice(kb * 128, (kb + 1) * 128)
                                    nc.tensor.matmul(
                                        aq[:, 2 * j, :], lhsT=K1z[h][:, ksl],
                                        rhs=QT[h][:, qsl], start=True,
                                        stop=True)
                                    nc.tensor.matmul(
                                        aq[:, 2 * j + 1, :],
                                        lhsT=K2z[h][:, ksl],
                                        rhs=QT[h][:, qsl], start=True,
                                        stop=True)
                                s_t = spool.tile([128, 4, QCH], F32R,
                                                 name="squad", tag="squad")
                                nc.scalar.activation(
                                    s_t[:], aq[:],
                                    mybir.ActivationFunctionType.Exp,
                                    scale=SCALE)
                                if s_prev is not None:
                                    gp = g - 1
                                    for j in range(2):
                                        kb = 2 * gp + j
                                        vsl = slice(kb * 65, (kb + 1) * 65)
                                        nc.tensor.matmul(
                                            o1[:], lhsT=Vex1[h][:, vsl],
                                            rhs=s_prev[:, 2 * j, :],
                                            start=(kb == 0), stop=False)
                                        nc.tensor.matmul(
                                            o2[:], lhsT=Vex2[h][:, vsl],
                                            rhs=s_prev[:, 2 * j + 1, :],
                                            start=(kb == 0), stop=False)
                                s_prev = s_t
                            for j in range(2):
                                kb = 2 * (NG - 1) + j
                                vsl = slice(kb * 65, (kb + 1) * 65)
                                nc.tensor.matmul(
                                    o1[:], lhsT=Vex1[h][:, vsl],
                                    rhs=s_prev[:, 2 * j, :], start=False,
                                    stop=(kb == NKB - 1))
                                nc.tensor.matmul(
                                    o2[:], lhsT=Vex2[h][:, vsl],
                                    rhs=s_prev[:, 2 * j + 1, :], start=False,
                                    stop=(kb == NKB - 1))

                            # epilogue head: copy off PSUM, reciprocal, cast
                            usb = epi.tile([65, 2 * QCH], F32, name="usb")
                            nc.vector.tensor_copy(usb[:, 0:QCH], o1[:])
                            nc.vector.tensor_copy(usb[:, QCH:2 * QCH], o2[:])
                            r32 = epi.tile([1, 2 * QCH], F32, name="r32")
                            nc.vector.reciprocal(r32[:], usb[64:65, :])
                            rr = epi.tile([1, 2 * QCH], F32R, name="rr")
                            nc.vector.tensor_copy(rr[:], r32[:])
                            pending = (h, qc, usb, rr)
                    emit_tail(pending)
                    pending = None

                # ---------- Y + LayerNorm ----------
                # h0 contraction half can run while h1's AllToAll is in flight
                with tc.tile_pool(name="psY", bufs=1, space="PSUM") as psY:
                    pY = {}
                    for sb_i in range(SS // 128):
                        ssl = slice(sb_i * 128, (sb_i + 1) * 128)
                        for eh in range(2):
                            esl = slice(eh * 512, (eh + 1) * 512)
                            p = psY.tile([128, 512], F32,
                                         name=f"pY{sb_i}{eh}",
                                         tag=f"pY{sb_i}{eh}")
                            pY[(sb_i, eh)] = p
                            for cc in range(NCORES):
                                nc.tensor.matmul(
                                    p[:], lhsT=ocT[cc][0:D, ssl],
                                    rhs=wo_t[0:D, cc, esl],
                                    start=(cc == 0), stop=False)
                    for sb_i in range(SS // 128):
                        ssl = slice(sb_i * 128, (sb_i + 1) * 128)
                        for eh in range(2):
                            esl = slice(eh * 512, (eh + 1) * 512)
                            p = pY[(sb_i, eh)]
                            for cc in range(NCORES):
                                nc.tensor.matmul(
                                    p[:], lhsT=ocT[cc][D:2 * D, ssl],
                                    rhs=wo_t[D:2 * D, cc, esl],
                                    start=False, stop=(cc == NCORES - 1))

                    for sb_i in range(SS // 128):
                        ssl = slice(sb_i * 128, (sb_i + 1) * 128)
                        ysb = epi.tile([128, E], F32, name="ysb")
                        for eh in range(2):
                            esl = slice(eh * 512, (eh + 1) * 512)
                            nc.vector.tensor_add(ysb[:, esl],
                                                 pY[(sb_i, eh)][:],
                                                 bo_b[:, esl])
                        # LayerNorm over E
                        st1 = epi.tile([128, 2, 6], F32, name="st1")
                        for g in range(2):
                            nc.vector.bn_stats(
                                out=st1[:, g, :],
                                in_=ysb[:, g * 512:(g + 1) * 512])
                        mv = epi.tile([128, 2], F32, name="mv")
                        nc.vector.bn_aggr(out=mv[:], in_=st1[:])
                        sd = epi.tile([128, 1], F32, name="sd")
                        nc.scalar.activation(
                            sd[:], mv[:, 1:2],
                            mybir.ActivationFunctionType.Sqrt,
                            bias=eps_t[:], scale=1.0)
                        rstd = epi.tile([128, 1], F32, name="rstd")
                        nc.vector.reciprocal(rstd[:], sd[:])
                        nrm = epi.tile([128, E], F32, name="nrm")
                        nc.vector.tensor_scalar(
                            out=nrm[:], in0=ysb[:], scalar1=mv[:, 0:1],
                            scalar2=rstd[:], op0=mybir.AluOpType.subtract,
                            op1=mybir.AluOpType.mult)
                        nc.vector.tensor_mul(nrm[:], nrm[:], gb2[:])
                        nc.vector.tensor_add(nrm[:], nrm[:], bb2[:])
                        nc.sync.dma_start(out=YOUT[ssl, :], in_=nrm[:])

    _split_excess_waits(nc)
    return nc


def kernel(X, Wq, bq, Wk, bk, Wv, bv, Wo, bo, gamma, beta, lam, **trace_kwargs):
    X = np.asarray(X)
    Wq = np.asarray(Wq)
    bq = np.asarray(bq)
    Wk = np.asarray(Wk)
    bk = np.asarray(bk)
    Wv = np.asarray(Wv)
    bv = np.asarray(bv)
    Wo = np.asarray(Wo)
    bo = np.asarray(bo)
    gamma = np.asarray(gamma)
    beta = np.asarray(beta)
    lam = np.asarray(lam)

    if "prog" not in _CACHE:
        _CACHE["prog"] = build_program()
    nc = _CACHE["prog"]

    Xf = np.ascontiguousarray(X.reshape(S, E).astype(np.float32))
    ident = np.eye(128, dtype=np.float32)
    onesrow = np.ones((1, 64), dtype=np.float32)
    onescol = np.ones((128, 1), dtype=np.float32)

    in_maps = []
    for i in range(NCORES):
        hs = slice(i * HC, (i + 1) * HC)
        in_maps.append({
            "X": Xf,
            "WQ": np.ascontiguousarray(Wq[hs].astype(np.float32)),
            "WK": np.ascontiguousarray(Wk[hs].astype(np.float32)),
            "WV": np.ascontiguousarray(Wv[hs].astype(np.float32)),
            "BQ": np.ascontiguousarray(bq[hs].astype(np.float32)),
            "BK": np.ascontiguousarray(bk[hs].astype(np.float32)),
            "BV": np.ascontiguousarray(bv[hs].astype(np.float32).reshape(-1)),
            "WO": np.ascontiguousarray(Wo.astype(np.float32)),
            "BO": np.ascontiguousarray(bo.astype(np.float32)),
            "GAMMA": np.ascontiguousarray(gamma.astype(np.float32)),
            "BETA": np.ascontiguousarray(beta.astype(np.float32)),
            "LAM": np.asarray([float(lam)], dtype=np.float32),
            "IDENT": ident,
            "ONESROW": onesrow,
            "ONESCOL": onescol,
        })

    res = run_bass_kernel_spmd(nc, in_maps, core_ids=list(range(NCORES)),
                               **trace_kwargs)
    _CACHE["last_result"] = res
    out = np.concatenate([res.results[i]["YOUT"] for i in range(NCORES)], axis=0)
    return out.reshape(B, S, E)


# revision 8
# speedup vs baseline: 1.1118x; 1.1118x over previous
"""Multi-head differential attention on 8 TRN2 NeuronCores.

Strategy (tensor parallel over heads + sequence-parallel epilogue):
  - Each core owns 2 of the 16 heads. It computes Q/K/V projections for its
    heads (from a shared on-chip X^T), flash-style attention entirely on-chip
    in transposed layouts, and the per-head differential-softmax combine.
  - An AllToAll (1 MB/rank) redistributes the combined per-head outputs
    Ocat^T [c, q] from head-sharding to sequence-sharding.
  - Each core then does its 256-row slice of Y = Ocat @ Wo + bo, LayerNorm,
    and the final (1 - lambda_init) scale. Host concatenates the 8 slices.

All matmuls run in float32r (full PE rate at >=256 free dim, ~13-bit mantissa);
exp/softmax/LN math in fp32 on ACT/DVE.
"""
import numpy as np

import concourse.bass as bass
import concourse.mybir as mybir
import concourse.tile as tile
from concourse.bass_utils import run_bass_kernel_spmd
import bass_rust

F32 = mybir.dt.float32
F32R = mybir.dt.float32r

B, S, E, H, D = 1, 2048, 1024, 16, 64
NCORES = 8
HC = H // NCORES          # heads per core = 2
SS = S // NCORES          # sequence shard = 256
C2D = 2 * D               # 128
LAM_INIT = 0.8
LN_EPS = 1e-5
QCH = 512                 # q-chunk width in the attention loop
NQC = S // QCH            # 4
NKB = S // 128            # 16 k-blocks
NEC = E // 128            # 8 contraction chunks
SCALE = 1.0 / float(np.sqrt(D))

_CACHE = {}


def _split_excess_waits(nc, max_waits=1):
    """walrus in this env only accepts ~1 sync wait per instruction; move the
    rest onto same-engine InstNoOp carriers emitted just before."""
    for bb in nc.main_func.blocks:
        new_list = []
        for ins in bb.instructions:
            w = list(ins.sync_info.on_wait) if ins.sync_info else []
            if len(w) > max_waits:
                extra, keep = w[:-max_waits], w[-max_waits:]
                for i, sw in enumerate(extra):
                    nop = mybir.InstNoOp(
                        name=f"{ins.name}-ps{i}", ins=[], outs=[],
                        engine=ins.engine,
                        sync_info=bass_rust.SyncInfo(on_wait=[sw], on_update=[]),
                    )
                    nc.register_instruction(nop)
                    new_list.append(nop)
                ins.sync_info = bass_rust.SyncInfo(
                    on_wait=keep, on_update=list(ins.sync_info.on_update))
            new_list.append(ins)
        bb.instructions[:] = new_list


def _dram_bcast(ap, parts):
    """Partition-broadcast read AP over an ExternalInput DRAM tensor."""
    return bass.AP(tensor=ap.tensor, offset=ap.offset,
                   ap=[[0, parts]] + [list(d) for d in ap.ap])


def build_program():
    nc = bass.Bass()

    X = nc.declare_dram_parameter("X", [S, E], F32R, isOutput=False)
    WQ = nc.declare_dram_parameter("WQ", [HC, E, C2D], F32R, isOutput=False)
    WK = nc.declare_dram_parameter("WK", [HC, E, C2D], F32R, isOutput=False)
    WV = nc.declare_dram_parameter("WV", [HC, E, D], F32R, isOutput=False)
    BQ = nc.declare_dram_parameter("BQ", [HC, C2D], F32, isOutput=False)
    BK = nc.declare_dram_parameter("BK", [HC, C2D], F32, isOutput=False)
    BV = nc.declare_dram_parameter("BV", [HC * D], F32, isOutput=False)
    WO = nc.declare_dram_parameter("WO", [E, E], F32R, isOutput=False)
    BO = nc.declare_dram_parameter("BO", [E], F32, isOutput=False)
    GAMMA = nc.declare_dram_parameter("GAMMA", [E], F32, isOutput=False)
    BETA = nc.declare_dram_parameter("BETA", [E], F32, isOutput=False)
    LAM = nc.declare_dram_parameter("LAM", [1], F32, isOutput=False)
    IDENT = nc.declare_dram_parameter("IDENT", [128, 128], F32R, isOutput=False)
    ONESROW = nc.declare_dram_parameter("ONESROW", [1, 64], F32R, isOutput=False)
    ONESCOL = nc.declare_dram_parameter("ONESCOL", [128, 1], F32R, isOutput=False)
    YOUT = nc.declare_dram_parameter("YOUT", [SS, E], F32, isOutput=True)

    with tile.TileContext(nc) as tc:
        with (
            tc.tile_pool(name="singles", bufs=1) as singles,
            tc.tile_pool(name="persist", bufs=1) as persist,
            tc.tile_pool(name="dram", bufs=1, space="DRAM") as dram,
        ):
            # ---------- constants ----------
            ident = singles.tile([128, 128], F32R)
            nc.sync.dma_start(out=ident[:], in_=IDENT[:, :])
            onesrow = singles.tile([1, 64], F32R)
            nc.gpsimd.dma_start(out=onesrow[:], in_=ONESROW[:, :])
            onescol = singles.tile([128, 1], F32R)
            nc.gpsimd.dma_start(out=onescol[:], in_=ONESCOL[:, :])
            lam_b = singles.tile([128, 1], F32)
            nc.gpsimd.dma_start(out=lam_b[:], in_=_dram_bcast(LAM[:], 128))
            gb2 = singles.tile([128, E], F32)
            nc.gpsimd.dma_start(out=gb2[:], in_=_dram_bcast(GAMMA[:], 128))
            bb2 = singles.tile([128, E], F32)
            nc.gpsimd.dma_start(out=bb2[:], in_=_dram_bcast(BETA[:], 128))
            bo_b = singles.tile([128, E], F32)
            nc.gpsimd.dma_start(out=bo_b[:], in_=_dram_bcast(BO[:], 128))
            # fold the final (1 - LAM_INIT) scale into gamma/beta broadcasts
            nc.vector.tensor_scalar_mul(gb2[:], gb2[:], 1.0 - LAM_INIT)
            nc.vector.tensor_scalar_mul(bb2[:], bb2[:], 1.0 - LAM_INIT)
            eps_t = singles.tile([128, 1], F32)
            nc.gpsimd.memset(eps_t[:], LN_EPS)

            bq_t, bk_t = [], []
            for h in range(HC):
                t = singles.tile([128, 1], F32, name=f"bq{h}", tag=f"bq{h}")
                nc.gpsimd.dma_start(out=t[:], in_=BQ[h, :].unsqueeze(1))
                bq_t.append(t)
                t = singles.tile([128, 1], F32, name=f"bk{h}", tag=f"bk{h}")
                nc.gpsimd.dma_start(out=t[:], in_=BK[h, :].unsqueeze(1))
                bk_t.append(t)
            bv_t = singles.tile([128, 1], F32)
            nc.gpsimd.dma_start(out=bv_t[:], in_=BV[:].unsqueeze(1))

            # long-lived attention operands
            QT = [persist.tile([128, S], F32R, name=f"QT{h}", tag=f"QT{h}")
                  for h in range(HC)]
            K1z = [persist.tile([128, S], F32R, name=f"K1z{h}", tag=f"K1z{h}")
                   for h in range(HC)]
            K2z = [persist.tile([128, S], F32R, name=f"K2z{h}", tag=f"K2z{h}")
                   for h in range(HC)]
            Vex1 = [persist.tile([128, NKB * 65], F32R, name=f"Vex1{h}",
                                 tag=f"Vex1{h}") for h in range(HC)]
            Vex2 = [persist.tile([128, NKB * 65], F32R, name=f"Vex2{h}",
                                 tag=f"Vex2{h}") for h in range(HC)]

            # one AllToAll per head so the first overlaps the second head's
            # attention compute
            a2a_in = [dram.tile([NCORES, D, SS], F32, name=f"a2ai{h}",
                                tag=f"a2ai{h}") for h in range(HC)]
            a2a_out = [dram.tile([NCORES, D, SS], F32, name=f"a2ao{h}",
                                 tag=f"a2ao{h}") for h in range(HC)]

            # ================= projection scope (xT lives here) =============
            with (
                tc.tile_pool(name="xtp", bufs=1) as xtp,
                tc.tile_pool(name="xstage", bufs=3) as xstage,
                tc.tile_pool(name="psT", bufs=4, space="PSUM") as psT,
                tc.tile_pool(name="psP", bufs=2, space="PSUM") as psP,
            ):
                # projection weights: [128 part (e within chunk), NEC, cols]
                wq_t = [xtp.tile([128, NEC, C2D], F32R, name=f"wq{h}",
                                 tag=f"wq{h}") for h in range(HC)]
                wk_t = [xtp.tile([128, NEC, C2D], F32R, name=f"wk{h}",
                                 tag=f"wk{h}") for h in range(HC)]
                for h in range(HC):
                    nc.gpsimd.dma_start(
                        out=wq_t[h][:],
                        in_=WQ[h].rearrange("(a p) c -> p a c", p=128))
                    nc.gpsimd.dma_start(
                        out=wk_t[h][:],
                        in_=WK[h].rearrange("(a p) c -> p a c", p=128))
                wv_t = xtp.tile([128, NEC, 2 * D], F32R)
                for h in range(HC):
                    nc.gpsimd.dma_start(
                        out=wv_t[:, :, h * D:(h + 1) * D],
                        in_=WV[h].rearrange("(a p) c -> p a c", p=128))

                # ----- X^T via PE transposes -----
                xT = [xtp.tile([128, S], F32R, name=f"xT{ec}", tag=f"xT{ec}")
                      for ec in range(NEC)]
                for sc in range(S // 128):
                    xs = xstage.tile([128, E], F32R, name="xs")
                    nc.sync.dma_start(out=xs[:],
                                      in_=X[sc * 128:(sc + 1) * 128, :])
                    for ec in range(NEC):
                        pst = psT.tile([128, 128], F32R, name="pst")
                        nc.tensor.transpose(
                            pst[:], xs[:, ec * 128:(ec + 1) * 128], ident[:])
                        nc.vector.tensor_copy(
                            xT[ec][:, sc * 128:(sc + 1) * 128], pst[:])

                # zero the unused halves of K1z/K2z once
                for h in range(HC):
                    nc.vector.tensor_scalar_mul(
                        K1z[h][64:128, :],
                        onescol[0:64, :].bitcast(F32).to_broadcast((64, S)), 0.0)
                    nc.vector.tensor_scalar_mul(
                        K2z[h][0:64, :],
                        onescol[0:64, :].bitcast(F32).to_broadcast((64, S)), 0.0)

                # ----- projections (transposed outputs) -----
                VTp = xtp.tile([128, S], F32R, name="VTp")

                def project(dst_writes, w_tile, qc):
                    pq = psP.tile([128, QCH], F32, name="pq")
                    for ec in range(NEC):
                        nc.tensor.matmul(
                            pq[:], lhsT=w_tile[:, ec, :],
                            rhs=xT[ec][:, qc * QCH:(qc + 1) * QCH],
                            start=(ec == 0), stop=(ec == NEC - 1))
                    dst_writes(pq)

                for qc in range(NQC):
                    sl = slice(qc * QCH, (qc + 1) * QCH)
                    for h in range(HC):
                        def wq_writes(pq, h=h, sl=sl):
                            nc.vector.tensor_scalar(
                                out=QT[h][:, sl], in0=pq[:],
                                scalar1=bq_t[h][:], scalar2=None,
                                op0=mybir.AluOpType.add)
                        project(wq_writes, wq_t[h], qc)

                        def wk_writes(pq, h=h, sl=sl):
                            nc.vector.tensor_scalar(
                                out=K1z[h][0:64, sl], in0=pq[0:64, :],
                                scalar1=bk_t[h][0:64, :], scalar2=None,
                                op0=mybir.AluOpType.add)
                            nc.vector.tensor_scalar(
                                out=K2z[h][64:128, sl], in0=pq[64:128, :],
                                scalar1=bk_t[h][64:128, :], scalar2=None,
                                op0=mybir.AluOpType.add)
                        project(wk_writes, wk_t[h], qc)

                    def wv_writes(pq, sl=sl):
                        nc.vector.tensor_scalar(
                            out=VTp[:, sl], in0=pq[:], scalar1=bv_t[:],
                            scalar2=None, op0=mybir.AluOpType.add)
                    project(wv_writes, wv_t, qc)

                # ----- V natural + [V|1] / [lam*V|1] -----
                for h in range(HC):
                    for vx in (Vex1[h], Vex2[h]):
                        nc.vector.tensor_copy(
                            vx.rearrange("p (k c) -> p k c", c=65)[:, :, 64:65],
                            onescol[:].to_broadcast((128, NKB)).unsqueeze(2))
                for kb in range(NKB):
                    pvt = psT.tile([128, 128], F32R, name="pvt", tag="pst")
                    nc.tensor.transpose(
                        pvt[:], VTp[:, kb * 128:(kb + 1) * 128], ident[:])
                    for h in range(HC):
                        nc.vector.tensor_copy(
                            Vex1[h][:, kb * 65:kb * 65 + 64],
                            pvt[:, h * D:(h + 1) * D])
                        nc.vector.tensor_scalar(
                            out=Vex2[h][:, kb * 65:kb * 65 + 64],
                            in0=pvt[:, h * D:(h + 1) * D], scalar1=lam_b[:],
                            scalar2=None, op0=mybir.AluOpType.mult)
            # ============== end projection scope (xT space freed) ===========

            with (
                tc.tile_pool(name="phase4", bufs=1) as phase4,
                tc.tile_pool(name="spool", bufs=3) as spool,
                tc.tile_pool(name="epi", bufs=2) as epi,
            ):
                # Wo moving operand, loaded during attention:
                # [128 part (c within chunk), NEC, E]
                wo_t = phase4.tile([128, NEC, E], F32R)
                nc.gpsimd.dma_start(
                    out=wo_t[:], in_=WO[:, :].rearrange("(a p) e -> p a e",
                                                        p=128))

                # ---------- attention per (head, q-chunk) ----------
                NG = NKB // 2  # 2 k-blocks per exp group

                def emit_a2a(h):
                    nc.gpsimd.collective_compute(
                        "AllToAll", mybir.AluOpType.bypass,
                        replica_groups=[list(range(NCORES))],
                        ins=[a2a_in[h][:, :, :]],
                        outs=[a2a_out[h][:, :, :]],
                    )

                ocT = [phase4.tile([128, SS], F32R, name=f"ocT{cc}",
                                   tag=f"ocT{cc}") for cc in range(NCORES)]
                with (
                    tc.tile_pool(name="psA", bufs=2, space="PSUM") as psA,
                    tc.tile_pool(name="psO", bufs=2, space="PSUM") as psO,
                    tc.tile_pool(name="psB", bufs=1, space="PSUM") as psB,
                ):
                    pending = None  # deferred epilogue tail of prev iteration

                    def emit_tail(p):
                        h, qc, usb, rr = p
                        bp = psB.tile([64, 2 * QCH], F32, name="bp")
                        nc.tensor.matmul(bp[:, 0:QCH], lhsT=onesrow[:],
                                         rhs=rr[:, 0:QCH], start=True,
                                         stop=True)
                        nc.tensor.matmul(bp[:, QCH:2 * QCH], lhsT=onesrow[:],
                                         rhs=rr[:, QCH:2 * QCH], start=True,
                                         stop=True)
                        bsb = epi.tile([64, 2 * QCH], F32, name="bsb")
                        nc.vector.tensor_copy(bsb[:], bp[:])
                        tmp = epi.tile([64, QCH], F32, name="tmpc")
                        nc.vector.tensor_mul(tmp[:], usb[0:64, QCH:2 * QCH],
                                             bsb[:, QCH:2 * QCH])
                        oc = epi.tile([64, QCH], F32, name="oc")
                        nc.vector.tensor_mul(oc[:], usb[0:64, 0:QCH],
                                             bsb[:, 0:QCH])
                        nc.vector.tensor_sub(oc[:], oc[:], tmp[:])
                        # stage into a2a_in: q-chunk covers dests 2qc, 2qc+1
                        for half in range(2):
                            dest = 2 * qc + half
                            nc.gpsimd.dma_start(
                                out=a2a_in[h][dest, :, :],
                                in_=oc[:, half * SS:(half + 1) * SS])
                        if qc == NQC - 1:
                            emit_a2a(h)
                            for cc in range(NCORES):
                                nc.gpsimd.dma_start(
                                    out=ocT[cc][h * D:(h + 1) * D, :],
                                    in_=a2a_out[h][cc, :, :])

                    for h in range(HC):
                        for qc in range(NQC):
                            qsl = slice(qc * QCH, (qc + 1) * QCH)
                            o1 = psO.tile([65, QCH], F32, name="oacc",
                                          tag="oacc")
                            o2 = psO.tile([65, QCH], F32, name="oacc2",
                                          tag="oacc")
                            s_prev = None
                            for kb in range(NKB):
                                if kb == NKB - 4 and pending is not None:
                                    emit_tail(pending)
                                    pending = None
                                ksl = slice(kb * 128, (kb + 1) * 128)
                                aq = psA.tile([128, 2, QCH], F32, name="aq",
                                              tag="aq")
                                nc.tensor.matmul(
                                    aq[:, 0, :], lhsT=K1z[h][:, ksl],
                                    rhs=QT[h][:, qsl], start=True, stop=True)
                                nc.tensor.matmul(
                                    aq[:, 1, :], lhsT=K2z[h][:, ksl],
                                    rhs=QT[h][:, qsl], start=True, stop=True)
                                s_t = spool.tile([128, 2, QCH], F32R,
                                                 name="spair", tag="spair")
                                nc.scalar.activation(
                                    s_t[:], aq[:],
                                    mybir.ActivationFunctionType.Exp,
                                    scale=SCALE)
                                if s_prev is not None:
                                    kp = kb - 1
                                    vsl = slice(kp * 65, (kp + 1) * 65)
                                    nc.tensor.matmul(
                                        o1[:], lhsT=Vex1[h][:, vsl],
                                        rhs=s_prev[:, 0, :],
                                        start=(kp == 0), stop=False)
                                    nc.tensor.matmul(
                                        o2[:], lhsT=Vex2[h][:, vsl],
                                        rhs=s_prev[:, 1, :],
                                        start=(kp == 0), stop=False)
                                s_prev = s_t
                            kp = NKB - 1
                            vsl = slice(kp * 65, (kp + 1) * 65)
                            nc.tensor.matmul(
                                o1[:], lhsT=Vex1[h][:, vsl],
                                rhs=s_prev[:, 0, :], start=False,
                                stop=True)
                            nc.tensor.matmul(
                                o2[:], lhsT=Vex2[h][:, vsl],
                                rhs=s_prev[:, 1, :], start=False,
                                stop=True)

                            # epilogue head: copy off PSUM, reciprocal, cast
                            usb = epi.tile([65, 2 * QCH], F32, name="usb")
                            nc.vector.tensor_copy(usb[:, 0:QCH], o1[:])
                            nc.vector.tensor_copy(usb[:, QCH:2 * QCH], o2[:])
                            r32 = epi.tile([1, 2 * QCH], F32, name="r32")
                            nc.vector.reciprocal(r32[:], usb[64:65, :])
                            rr = epi.tile([1, 2 * QCH], F32R, name="rr")
                            nc.vector.tensor_copy(rr[:], r32[:])
                            pending = (h, qc, usb, rr)
                    emit_tail(pending)
                    pending = None

                # ---------- Y + LayerNorm ----------
                # h0 contraction half can run while h1's AllToAll is in flight
                with tc.tile_pool(name="psY", bufs=1, space="PSUM") as psY:
                    pY = {}
                    for sb_i in range(SS // 128):
                        ssl = slice(sb_i * 128, (sb_i + 1) * 128)
                        for eh in range(2):
                            esl = slice(eh * 512, (eh + 1) * 512)
                            p = psY.tile([128, 512], F32,
                                         name=f"pY{sb_i}{eh}",
                                         tag=f"pY{sb_i}{eh}")
                            pY[(sb_i, eh)] = p
                            for cc in range(NCORES):
                                nc.tensor.matmul(
                                    p[:], lhsT=ocT[cc][0:D, ssl],
                                    rhs=wo_t[0:D, cc, esl],
                                    start=(cc == 0), stop=False)
                    for sb_i in range(SS // 128):
                        ssl = slice(sb_i * 128, (sb_i + 1) * 128)
                        for eh in range(2):
                            esl = slice(eh * 512, (eh + 1) * 512)
                            p = pY[(sb_i, eh)]
                            for cc in range(NCORES):
                                nc.tensor.matmul(
                                    p[:], lhsT=ocT[cc][D:2 * D, ssl],
                                    rhs=wo_t[D:2 * D, cc, esl],
                                    start=False, stop=(cc == NCORES - 1))

                    for sb_i in range(SS // 128):
                        ssl = slice(sb_i * 128, (sb_i + 1) * 128)
                        ysb = epi.tile([128, E], F32, name="ysb")
                        for eh in range(2):
                            esl = slice(eh * 512, (eh + 1) * 512)
                            nc.vector.tensor_add(ysb[:, esl],
                                                 pY[(sb_i, eh)][:],
                                                 bo_b[:, esl])
                        # LayerNorm over E
                        st1 = epi.tile([128, 2, 6], F32, name="st1")
                        for g in range(2):
                            nc.vector.bn_stats(
                                out=st1[:, g, :],
                                in_=ysb[:, g * 512:(g + 1) * 512])
                        mv = epi.tile([128, 2], F32, name="mv")
                        nc.vector.bn_aggr(out=mv[:], in_=st1[:])
                        sd = epi.tile([128, 1], F32, name="sd")
                        nc.scalar.activation(
                            sd[:], mv[:, 1:2],
                            mybir.ActivationFunctionType.Sqrt,
                            bias=eps_t[:], scale=1.0)
                        rstd = epi.tile([128, 1], F32, name="rstd")
                        nc.vector.reciprocal(rstd[:], sd[:])
                        nrm = epi.tile([128, E], F32, name="nrm")
                        nc.vector.tensor_scalar(
                            out=nrm[:], in0=ysb[:], scalar1=mv[:, 0:1],
                            scalar2=rstd[:], op0=mybir.AluOpType.subtract,
                            op1=mybir.AluOpType.mult)
                        nc.vector.tensor_mul(nrm[:], nrm[:], gb2[:])
                        nc.vector.tensor_add(nrm[:], nrm[:], bb2[:])
                        nc.sync.dma_start(out=YOUT[ssl, :], in_=nrm[:])

    _split_excess_waits(nc)
    return nc


def kernel(X, Wq, bq, Wk, bk, Wv, bv, Wo, bo, gamma, beta, lam, **trace_kwargs):
    X = np.asarray(X)
    Wq = np.asarray(Wq)
    bq = np.asarray(bq)
    Wk = np.asarray(Wk)
    bk = np.asarray(bk)
    Wv = np.asarray(Wv)
    bv = np.asarray(bv)
    Wo = np.asarray(Wo)
    bo = np.asarray(bo)
    gamma = np.asarray(gamma)
    beta = np.asarray(beta)
    lam = np.asarray(lam)

    if "prog" not in _CACHE:
        _CACHE["prog"] = build_program()
    nc = _CACHE["prog"]

    Xf = np.ascontiguousarray(X.reshape(S, E).astype(np.float32))
    ident = np.eye(128, dtype=np.float32)
    onesrow = np.ones((1, 64), dtype=np.float32)
    onescol = np.ones((128, 1), dtype=np.float32)

    in_maps = []
    for i in range(NCORES):
        hs = slice(i * HC, (i + 1) * HC)
        in_maps.append({
            "X": Xf,
            "WQ": np.ascontiguousarray(Wq[hs].astype(np.float32)),
            "WK": np.ascontiguousarray(Wk[hs].astype(np.float32)),
            "WV": np.ascontiguousarray(Wv[hs].astype(np.float32)),
            "BQ": np.ascontiguousarray(bq[hs].astype(np.float32)),
            "BK": np.ascontiguousarray(bk[hs].astype(np.float32)),
            "BV": np.ascontiguousarray(bv[hs].astype(np.float32).reshape(-1)),
            "WO": np.ascontiguousarray(Wo.astype(np.float32)),
            "BO": np.ascontiguousarray(bo.astype(np.float32)),
            "GAMMA": np.ascontiguousarray(gamma.astype(np.float32)),
            "BETA": np.ascontiguousarray(beta.astype(np.float32)),
            "LAM": np.asarray([float(lam)], dtype=np.float32),
            "IDENT": ident,
            "ONESROW": onesrow,
            "ONESCOL": onescol,
        })

    res = run_bass_kernel_spmd(nc, in_maps, core_ids=list(range(NCORES)),
                               **trace_kwargs)
    _CACHE["last_result"] = res
    out = np.concatenate([res.results[i]["YOUT"] for i in range(NCORES)], axis=0)
    return out.reshape(B, S, E)
